# revision 58
# baseline (speedup 1.0000x reference)
"""Trainium2 Bass kernel for nn_Encoder_Block (B=2,S=2048,E=1024,H=16,D=64,FE=4).

Sharding: 8 NeuronCores, no collectives. Cores 0-3 take batch 0, cores 4-7
batch 1; each core owns a 512-query slice and runs the full encoder block
for those queries (it loads all keys/values of its batch plus all weights).

Numerics: fp16 two-term split replaces the fp32r hi/lo scheme. With
k = k1 + k2 and q' = q1 + q2 (fp16 parts), scores ~= k1*q1 + k1*q2 + k2*q1
(the dropped k2*q2 term is ~2^-22 relative). Pass 2 computes, per k-tile,
  mm1: lhsT=[k1;k1] rhs=[q1;q2]  -> k1*(q1+q2)
  mm2: lhsT=[k2;1]  rhs=[q1;-m]  -> k2*q1 - m
so operands DMA straight into fp16 tiles (no DVE rounding copies).

Per-core pipeline, software-pipelined across heads: in iteration h the
program issues DMA loads for head h+2, pass-1 (max) matmuls+reduces for
head h+1, then pass-2/exp/ov for head h. PE therefore never waits on the
max-reduction chain, which runs on DVE and the otherwise-idle Pool engine
in alternation. Then fc (Wv folded into Wfc), residual + LN1 (bn_stats),
FFN1 + relu(+b1 via ACT bias), FFN2 (+b2 via K=1 matmul), residual + LN2.
Weights pre-transposed / pre-cast / pre-tiled on the host.
"""
import os
import sys
import math
from contextlib import ExitStack

os.environ.setdefault("NEURON_RT_RESET_CORES", "1")
sys.path.insert(0, "/opt/trn_rl_repo")

import numpy as np
import concourse.bass as bass
import concourse.tile as tile
from concourse import mybir

F32 = mybir.dt.float32
F16 = mybir.dt.float16
F8 = mybir.dt.float8e4
AX = mybir.AxisListType.X
AF = mybir.ActivationFunctionType
OP = mybir.AluOpType


class Cfg:
    def __init__(self, S=2048, E=1024, H=16, D=64, FE=4, T=512, eps=1e-5):
        self.S, self.E, self.H, self.D, self.FE, self.T, self.eps = S, E, H, D, FE, T, eps
        assert D == 64 and E == H * D
        self.KT = S // 128            # k partition-tiles
        self.QT = T // 128            # q tiles (per core)
        self.ET = E // 128            # e tiles
        self.ZT = FE * E // 128       # ffn hidden tiles
        self.CH = min(512, S)         # k moving chunk for pass1
        self.NCH = S // self.CH
        self.EC = min(512, E)         # e moving chunk
        self.NEC = E // self.EC
        self.P2B = 2 if self.KT % 2 == 0 else 1   # pass-2 k-tiles per exp batch
        self.scale = math.sqrt(float(S))

    def perm(self):
        # pass-2 query order j <-> original query (j % QT)*128 + j // QT
        j = np.arange(self.T)
        return (j % self.QT) * 128 + j // self.QT


def _layernorm(nc, pool, x_ap, out_ap, g_b, b_b, eps_t, c):
    """LayerNorm over the free dim (E) of x_ap [128, E] (fp16) -> out_ap.

    Intermediates in fp16 (2x DVE) with per-partition f32 scalars (exempt
    from the 2x dtype rule)."""
    E = c.E
    nsub = (E + 511) // 512
    stats = pool.tile([128, nsub, 6], F32, tag="ln_stats")
    xr = x_ap.rearrange("p (n s) -> p n s", n=nsub)
    for i in range(nsub):
        nc.vector.bn_stats(stats[:, i, :], xr[:, i, :])
    mv = pool.tile([128, 2], F32, tag="ln_mv")
    nc.vector.bn_aggr(mv[:], stats[:])
    rstd = pool.tile([128, 1], F32, tag="ln_rstd")
    nc.scalar.activation(rstd[:], mv[:, 1:2], AF.Sqrt, bias=eps_t[:], scale=1.0)
    nc.vector.reciprocal(rstd[:], rstd[:])
    t1 = pool.tile([128, E], F16, tag="ln_t1")
    nc.vector.tensor_scalar(t1[:], x_ap, mv[:, 0:1], rstd[:],
                            OP.subtract, OP.mult)
    t2 = pool.tile([128, E], F16, tag="ln_t2")
    nc.vector.tensor_tensor(t2[:], t1[:], g_b[:], OP.mult)
    nc.vector.tensor_tensor(out_ap, t2[:], b_b[:], OP.add)


def build_nc(c: Cfg):
    """Build the single-core program (pure SPMD — all cores run this)."""
    nc = bass.Bass()
    S, E, H, D, T = c.S, c.E, c.H, c.D, c.T

    dp = nc.declare_dram_parameter
    k1_d = dp("k1", [H, 64, S], F16, isOutput=False)         # k1^T per head
    k2o_d = dp("k2o", [H, 65, S], F16, isOutput=False)       # [k2^T; ones]
    q1o_d = dp("q1o", [E, T], F16, isOutput=False)           # q1'^T orig order
    qq_d = dp("qq", [H, 128, T], F16, isOutput=False)        # [q1'^T; q2'^T] perm
    qnat_d = dp("qnat", [T, E], F16, isOutput=False)         # queries+bfc (perm)
    v_d = dp("vv", [H, 128, c.KT, 65], F16, isOutput=False)  # v_aug pre-tiled
    wfc_d = dp("wfc", [128, c.ET, E], F16, isOutput=False)   # Wfc_v^T tiled
    w1_d = dp("w1", [c.ZT, 128, E], F16, isOutput=False)     # per zt: [e part, z cols]
    b1_d = dp("b1", [128, c.ZT], F32, isOutput=False)
    w2_d = dp("w2", [c.ZT, 128, E], F16, isOutput=False)     # per zt: [z part, e cols]
    b2_d = dp("b2", [1, E], F16, isOutput=False)
    g1_d = dp("g1", [1, E], F16, isOutput=False)
    be1_d = dp("be1", [1, E], F16, isOutput=False)
    g2_d = dp("g2", [1, E], F16, isOutput=False)
    be2_d = dp("be2", [1, E], F16, isOutput=False)
    out_d = dp("out", [T, E], F16, isOutput=True)            # perm rows

    with tile.TileContext(nc) as tc, ExitStack() as ctx:
        persist = ctx.enter_context(tc.tile_pool(name="persist", bufs=1))

        from concourse.masks import make_identity
        ident = persist.tile([128, 128], F16)
        make_identity(nc, ident[:])

        eps_t = persist.tile([128, 1], F32)
        nc.vector.memset(eps_t[:], c.eps)

        ones_16 = persist.tile([1, 128], F16)
        nc.vector.memset(ones_16[:], 1.0)

        # persistent tiles; DMAs issued later (after the first head loads)
        # so the attention pipeline starts immediately.
        g1_b = persist.tile([128, E], F16, name="g1b", tag="g1b")
        be1_b = persist.tile([128, E], F16, name="be1b", tag="be1b")
        g2_b = persist.tile([128, E], F16, name="g2b", tag="g2b")
        be2_b = persist.tile([128, E], F16, name="be2b", tag="be2b")
        wfc_t = persist.tile([128, c.ET, E], F16)
        b1_t = persist.tile([128, c.ZT], F32)
        b2_t = persist.tile([1, E], F16)

        def load_persist():
            for t, src in ((g1_b, g1_d), (be1_b, be1_d),
                           (g2_b, g2_d), (be2_b, be2_d)):
                src_ap = src[:]
                nc.sync.dma_start(
                    t[:], bass.AP(tensor=src_ap.tensor, offset=src_ap.offset,
                                  ap=[[0, 128]] + list(src_ap.ap[1:])))
            nc.sync.dma_start(wfc_t[:], wfc_d[:])
            nc.sync.dma_start(b1_t[:], b1_d[:])
            nc.sync.dma_start(b2_t[:], b2_d[:])

        # per-dt/per-et tiles (not one big tile) so downstream matmuls only
        # wait on the slices they actually read (tile-granular deps)
        ovT_pack = [persist.tile([128, T], F16, name=f"ovT{d}", tag=f"ovT{d}")
                    for d in range(c.ET)]
        h_sb = persist.tile([128, c.QT, E], F16)
        hT_16 = [persist.tile([128, T], F16, name=f"hT{e}", tag=f"hT{e}")
                 for e in range(c.ET)]
        x_sb = persist.tile([128, c.QT, E], F16)

        # =================== ATTENTION ===================
        with ExitStack() as actx:
            k1_p = actx.enter_context(tc.tile_pool(name="k1p", bufs=4))
            k2_p = actx.enter_context(tc.tile_pool(name="k2p", bufs=4))
            q_p = actx.enter_context(tc.tile_pool(name="qp", bufs=4))
            qa_p = actx.enter_context(tc.tile_pool(name="qap", bufs=4))
            vv_p = actx.enter_context(tc.tile_pool(name="vv", bufs=4))
            ovs_p = actx.enter_context(tc.tile_pool(name="ovs", bufs=2))
            attn_p = actx.enter_context(tc.tile_pool(name="attn", bufs=2))
            sm_p = actx.enter_context(tc.tile_pool(name="sm", bufs=3))
            zi_p = actx.enter_context(tc.tile_pool(name="zi", bufs=2))
            zdr_p = actx.enter_context(tc.tile_pool(name="zdr", bufs=2, space="DRAM"))
            mm_ps = actx.enter_context(tc.tile_pool(name="mm_ps", bufs=2, space="PSUM"))
            p2_ps = actx.enter_context(tc.tile_pool(name="p2_ps", bufs=2, space="PSUM"))
            ov_ps = actx.enter_context(tc.tile_pool(name="ov_ps", bufs=2, space="PSUM"))

            heads = [dict() for _ in range(H)]
            red_ctr = [0]

            def load_head(h):
                st = heads[h]
                # pass-1 operands (q1o, k1 hi half) first: the first matmul
                # of pass1(h) only waits on these two
                q1o = q_p.tile([64, T], F16, tag="q1o")
                nc.sync.dma_start(q1o[:], q1o_d[h * D:(h + 1) * D, :])
                k1 = k1_p.tile([128, S], F16, tag="k1")
                nc.sync.dma_start(k1[:64, :], k1_d[h, :, :])
                nc.sync.dma_start(k1[64:, :], k1_d[h, :, :])
                k2o = k2_p.tile([65, S], F16, tag="k2o")
                nc.sync.dma_start(k2o[:], k2o_d[h, :, :])
                qq = q_p.tile([128, T], F16, tag="qq")
                nc.sync.dma_start(qq[:], qq_d[h, :, :])
                vaug = vv_p.tile([128, c.KT, 65], F16, tag="vaug")
                nc.sync.dma_start(vaug[:], v_d[h, :, :, :])
                st["k1"], st["k2o"], st["q1o"], st["qq"], st["vaug"] = \
                    k1, k2o, q1o, qq, vaug

            def pass1_gen(h):
                """Row max of scores (fp16 hi-only), -m bounce, qaug tile.

                Reduces run on DVE (GPSIMD/Pool cannot read PSUM on TRN2
                hardware). Yields after every few chunk-matmuls so the caller
                can interleave them between pass-2 matmul groups (the PE
                queue is in-order; un-interleaved, pass-1 stalls PE on its
                own reduce-gated PSUM bank rotation)."""
                st = heads[h]
                k1, q1o, qq = st["k1"], st["q1o"], st["qq"]
                m16 = sm_p.tile([128, c.QT], F16, tag="m16")
                emitted = 0
                for qt in range(c.QT):
                    mtmp = sm_p.tile([128, max(c.NCH, 2)], F32,
                                     tag=f"mtmp{qt % 2}")
                    for j in range(c.NCH):
                        sl = slice(j * c.CH, (j + 1) * c.CH)
                        sps = mm_ps.tile([128, c.CH], F32, tag="mmps")
                        nc.tensor.matmul(
                            sps[:], q1o[:, qt * 128:(qt + 1) * 128],
                            k1[:64, sl], start=True, stop=True)
                        nc.vector.reduce_max(mtmp[:, j:j + 1], sps[:], axis=AX)
                        emitted += 1
                        if emitted % 3 == 0:
                            yield
                    # final: -max over NCH then cast fp16
                    nc.vector.tensor_reduce(
                        m16[:, qt:qt + 1], mtmp[:, :c.NCH], axis=AX,
                        op=OP.max, negate=True)
                m_dram = zdr_p.tile([128, c.QT], F16, tag="mdram")
                nc.sync.dma_start(m_dram[:], m16[:])
                # qaug [65, T]: rows 0-63 = q1' (perm), row 64 = -m
                qaug = qa_p.tile([65, T], F16, tag="qaug")
                nc.sync.dma_start(qaug[:64, :], qq[:64, :])
                nc.sync.dma_start(qaug[64:65, :],
                                  m_dram[:].rearrange("r qt -> (r qt)")[None, :])
                st["qaug"] = qaug

            def run_gen(gen):
                if gen is not None:
                    for _ in gen:
                        pass

            def pass2(h, ovst, p1gen=None):
                """ScoresT - m, exp, ov accumulate, 1/Z scale into ovst.
                Interleaves up to 2 pass-1 chunk-matmuls of the NEXT head
                between tb-groups (see pass1_gen)."""
                st = heads[h]
                k1, k2o, qq, qaug, vaug = \
                    st["k1"], st["k2o"], st["qq"], st["qaug"], st["vaug"]
                attnT = attn_p.tile([128, c.KT, T], F16, tag="attnT")
                ovp = ov_ps.tile([65, T], F32, tag="ovps")
                for tb in range(0, c.KT, c.P2B):
                    p2 = p2_ps.tile([128, c.P2B, T], F32, tag="p2ps")
                    for ti in range(c.P2B):
                        t = tb + ti
                        tsl = slice(t * 128, (t + 1) * 128)
                        # k1*(q1+q2)  then  k2*q1 + (-m)
                        nc.tensor.matmul(p2[:, ti, :], k1[:, tsl], qq[:],
                                         start=True, stop=False)
                        nc.tensor.matmul(p2[:, ti, :], k2o[:, tsl], qaug[:],
                                         start=False, stop=True)
                    nc.scalar.activation(attnT[:, tb:tb + c.P2B, :], p2[:],
                                         AF.Exp, bias=0.0, scale=c.scale)
                    for ti in range(c.P2B):
                        t = tb + ti
                        nc.tensor.matmul(
                            ovp[:], vaug[:, t, :], attnT[:, t, :],
                            start=(t == 0), stop=(t == c.KT - 1),
                            skip_group_check=True)
                    if p1gen is not None:
                        next(p1gen, None)

                # ---- 1/Z broadcast (DRAM bounce) and ovT drain ----
                zrow = zi_p.tile([1, T], F16, tag="zrow")
                with nc.allow_low_precision(reason="1/Z scale, 5e-4 rel ok"):
                    nc.vector.reciprocal(zrow[:], ovp[64:65, :])
                zdr = zdr_p.tile([1, T], F16, tag="zdr")
                nc.sync.dma_start(zdr[:], zrow[:])
                zinv_b = zi_p.tile([64, T], F16, tag="zinv")
                zsrc = zdr[:]
                nc.sync.dma_start(
                    zinv_b[:],
                    bass.AP(tensor=zsrc.tensor, offset=zsrc.offset,
                            ap=[[0, 64]] + list(zsrc.ap[1:])))
                if h % 2 == 0:
                    # even head: write partitions 0-63 of ovT_pack directly
                    nc.vector.scalar_tensor_tensor(
                        ovT_pack[h // 2][:64, :], ovp[:64, :], 1.0, zinv_b[:],
                        OP.bypass, OP.mult)
                else:
                    nc.vector.scalar_tensor_tensor(
                        ovst[:], ovp[:64, :], 1.0, zinv_b[:],
                        OP.bypass, OP.mult)

            # software-pipelined head loop: loads 3 ahead, pass1 2 ahead
            # (interleaved into pass2's matmul stream), so the -m DMA bounce
            # of head h+2 completes a full head before pass2(h+2) needs it.
            import itertools
            load_head(0)
            load_head(1)
            load_head(2)
            load_persist()   # fc/ffn constants stream behind the head loads
            run_gen(pass1_gen(0))
            ovst = None
            for h in range(H):
                if h % 2 == 1:
                    ovst = ovs_p.tile([64, T], F16, tag="ovst")
                if h + 3 < H:
                    load_head(h + 3)
                if h == 0:
                    # catch up: both pass1(1) and pass1(2) ride pass2(0)
                    gen = itertools.chain(pass1_gen(1), pass1_gen(2))
                else:
                    gen = pass1_gen(h + 2) if h + 2 < H else None
                pass2(h, ovst, gen)
                run_gen(gen)
                heads[h].clear()
                if h % 2 == 1:
                    # odd head: DMA into partitions 64-127
                    nc.sync.dma_start(ovT_pack[h // 2][64:128, :], ovst[:])

        # =================== FC + LN1 + transpose(h) ===================
        with ExitStack() as fctx:
            qn_p = fctx.enter_context(tc.tile_pool(name="qn", bufs=2))
            st_p = fctx.enter_context(tc.tile_pool(name="st", bufs=2))
            fc_ps = fctx.enter_context(tc.tile_pool(name="fc_ps", bufs=1, space="PSUM"))
            tr_ps = fctx.enter_context(tc.tile_pool(name="tr_ps", bufs=4, space="PSUM"))

            def transposes(qt):
                # PE transpose; PSUM->SBUF drains on ACT (DVE is busy with
                # LN1; Pool cannot read PSUM on TRN2 hardware)
                qsl = slice(qt * 128, (qt + 1) * 128)
                for et in range(c.ET):
                    tps = tr_ps.tile([128, 128], F16, tag="trps")
                    nc.tensor.transpose(tps[:],
                                        h_sb[:, qt, et * 128:(et + 1) * 128],
                                        ident[:])
                    nc.scalar.copy(hT_16[et][:, qsl], tps[:])

            # Two qt at a time (4 PSUM groups): dt 0..6 of all four groups
            # first — these don't depend on the last head pair, so they run
            # during the final attention Z-chain — then the dt=7 closers.
            hpres = {}
            for qt in range(c.QT):
                hpres[qt] = st_p.tile([128, E], F16, name=f"hpre{qt % 2}",
                                      tag=f"hpre{qt % 2}")
            for qt0 in range(0, c.QT, 2):
                groups = [(qt, ec) for qt in (qt0, qt0 + 1)
                          for ec in range(c.NEC)]
                qns = {}
                for qt in (qt0, qt0 + 1):
                    qn = qn_p.tile([128, E], F16, name=f"qn{qt % 2}",
                                   tag=f"qn{qt % 2}")
                    nc.sync.dma_start(qn[:], qnat_d[qt * 128:(qt + 1) * 128, :])
                    qns[qt] = qn
                aps = {}
                for qt, ec in groups:
                    qsl = slice(qt * 128, (qt + 1) * 128)
                    esl = slice(ec * c.EC, (ec + 1) * c.EC)
                    a = fc_ps.tile([128, c.EC], F32, name=f"fcps{qt % 2}_{ec}",
                                   tag=f"fcps{qt % 2}_{ec}")
                    aps[(qt, ec)] = a
                    for dt in range(c.ET - 1):
                        nc.tensor.matmul(a[:], ovT_pack[dt][:, qsl],
                                         wfc_t[:, dt, esl],
                                         start=(dt == 0), stop=False)
                for qt, ec in groups:
                    qsl = slice(qt * 128, (qt + 1) * 128)
                    esl = slice(ec * c.EC, (ec + 1) * c.EC)
                    dt = c.ET - 1
                    nc.tensor.matmul(aps[(qt, ec)][:], ovT_pack[dt][:, qsl],
                                     wfc_t[:, dt, esl], start=False, stop=True)
                    nc.vector.scalar_tensor_tensor(
                        hpres[qt][:, esl], aps[(qt, ec)][:], 1.0,
                        qns[qt][:, esl], OP.bypass, OP.add)
                for qt in (qt0, qt0 + 1):
                    _layernorm(nc, st_p, hpres[qt][:], h_sb[:, qt, :],
                               g1_b, be1_b, eps_t, c)
                if qt0 > 0:
                    transposes(qt0 - 2)
                    transposes(qt0 - 1)
            transposes(c.QT - 2)
            transposes(c.QT - 1)

        # =================== FFN + LN2 ===================
        with ExitStack() as nctx:
            w_p = nctx.enter_context(tc.tile_pool(name="wstream", bufs=4))
            z_p = nctx.enter_context(tc.tile_pool(name="zrel", bufs=1))
            ln_p = nctx.enter_context(tc.tile_pool(name="lnp", bufs=2))
            z1_ps = nctx.enter_context(tc.tile_pool(name="z1_ps", bufs=3, space="PSUM"))
            x2_ps = nctx.enter_context(
                tc.tile_pool(name="x2_ps", bufs=2, space="PSUM"))

            # w2 fully resident in SBUF; its loads interleave with the w1
            # stream so FFN2 (qt-outer, LN2 in-loop) never waits on DMA.
            w2sb = z_p.tile([128, c.ZT, E], F16, tag="w2sb")
            z1rel = z_p.tile([128, c.ZT, T], F16, tag="z1rel")
            for zt in range(c.ZT):
                w1t = w_p.tile([128, E], F16, tag="w1t")
                nc.sync.dma_start(w1t[:], w1_d[zt, :, :])
                nc.sync.dma_start(w2sb[:, zt, :], w2_d[zt, :, :])
                zps = z1_ps.tile([128, T], F32, tag="z1ps")
                for et in range(c.ET):
                    nc.tensor.matmul(zps[:], w1t[:, et * 128:(et + 1) * 128],
                                     hT_16[et][:],
                                     start=(et == 0), stop=(et == c.ET - 1))
                nc.scalar.activation(z1rel[:, zt, :], zps[:], AF.Relu,
                                     bias=b1_t[:, zt:zt + 1], scale=1.0)

            for qt in range(c.QT):
                qsl = slice(qt * 128, (qt + 1) * 128)
                for ec in range(c.NEC):
                    esl = slice(ec * c.EC, (ec + 1) * c.EC)
                    xps = x2_ps.tile([128, c.EC], F32, tag="x2ps")
                    for zt in range(c.ZT):
                        nc.tensor.matmul(
                            xps[:], z1rel[:, zt, qsl], w2sb[:, zt, esl],
                            start=(zt == 0), stop=False)
                    nc.tensor.matmul(xps[:], ones_16[:, :128], b2_t[:, esl],
                                     start=False, stop=True)
                    nc.vector.scalar_tensor_tensor(
                        x_sb[:, qt, esl], xps[:], 1.0, h_sb[:, qt, esl],
                        OP.bypass, OP.add)
                outt = ln_p.tile([128, E], F16, tag="outt")
                _layernorm(nc, ln_p, x_sb[:, qt, :], outt[:], g2_b, be2_b, eps_t, c)
                nc.sync.dma_start(out_d[qsl, :], outt[:])

    return nc


def _fix_instpool(nc):
    """Re-expand InstPool input APs to the 5d form the Pool unit needs
    (tile scheduling's AP re-lowering collapses the size-1 dims)."""
    from concourse import ap_utils
    for f in nc.m.functions:
        for bb in f.blocks:
            for inst in bb.instructions:
                if type(inst).__name__ == "InstPool":
                    nd = len(inst.ins[0].ap)
                    if nd != 5:
                        inst.ins[0].ap = mybir.VecI64Pair(
                            ap_utils.expand_dims_ap(
                                inst.ins[0].ap, list(range(1, 6 - nd))))


def _split_waits(nc, maxw=1):
    """walrus in this toolchain only accepts 1 sync-wait per instruction on
    several formats; move excess waits onto preceding same-engine NoOps."""
    _fix_instpool(nc)
    ctr = 0
    for f in nc.m.functions:
        for bb in f.blocks:
            out = []
            for inst in bb.instructions:
                si = getattr(inst, "sync_info", None)
                if si is not None and si.on_wait and len(si.on_wait) > maxw:
                    waits = list(si.on_wait)
                    head, tail = waits[:-maxw], waits[-maxw:]
                    for i in range(0, len(head), maxw):
                        ctr += 1
                        out.append(mybir.InstNoOp(
                            name=f"waitsplit_{ctr}", engine=inst.engine,
                            ins=[], outs=[],
                            sync_info=mybir.SyncInfo(
                                on_wait=list(head[i:i + maxw]), on_update=[]),
                        ))
                    si.on_wait = tail
                out.append(inst)
            bb.instructions[:] = out


# ======================= host side =======================

def host_prep(c: Cfg, inputs, core):
    """Build the per-core input map (numpy only)."""
    B = inputs["queries"].shape[0]
    cores_per_batch = 8 // B if B <= 8 else 1
    b = core // cores_per_batch
    slot = core % cores_per_batch
    T = c.T
    perm = c.perm()

    q = np.asarray(inputs["queries"][b], np.float32)       # [S, E]
    k = np.asarray(inputs["keys"][b], np.float32)
    v = np.asarray(inputs["values"][b], np.float32)
    qs = q[slot * T:(slot + 1) * T]                        # [T, E]

    Wq = np.asarray(inputs["Wq"], np.float64)
    Wk = np.asarray(inputs["Wk"], np.float64)
    Wv = np.asarray(inputs["Wv"], np.float64)
    Wfc = np.asarray(inputs["Wfc"], np.float64)            # [E, E]
    W1 = np.asarray(inputs["W1"], np.float64)              # [FE*E, E]
    W2 = np.asarray(inputs["W2"], np.float64)              # [E, FE*E]

    # fold Wq/Wk into the queries: q'_h = q_h @ (Wq.T @ Wk); scores = q' @ k^T
    A_mid = Wq.T @ Wk
    E_, H_, D_ = c.E, c.H, c.D
    qp = np.empty((T, E_), np.float64)
    for h in range(H_):
        qp[:, h * D_:(h + 1) * D_] = qs[:, h * D_:(h + 1) * D_].astype(np.float64) @ A_mid
    qp = qp.astype(np.float32)

    # fp16 two-term splits
    q1 = qp.astype(np.float16)
    q2 = (qp - q1.astype(np.float32)).astype(np.float16)
    k1 = k.astype(np.float16)
    k2 = (k - k1.astype(np.float32)).astype(np.float16)

    k1_prep = np.empty((c.H, 64, c.S), np.float16)
    k2o_prep = np.ones((c.H, 65, c.S), np.float16)
    for h in range(H_):
        k1_prep[h] = k1[:, h * D_:(h + 1) * D_].T
        k2o_prep[h, :64] = k2[:, h * D_:(h + 1) * D_].T

    q1oT = np.ascontiguousarray(q1.T)                      # [E, T] orig order
    q1p = q1[perm].T                                       # [E, T] perm order
    q2p = q2[perm].T
    qq_prep = np.empty((c.H, 128, T), np.float16)
    for h in range(H_):
        qq_prep[h, :64] = q1p[h * D_:(h + 1) * D_]
        qq_prep[h, 64:] = q2p[h * D_:(h + 1) * D_]

    # v_aug pre-tiled per head: [128, KT, 65] with ones column
    v16 = v.astype(np.float16)
    vv_prep = np.ones((c.H, 128, c.KT, 65), np.float16)
    for h in range(H_):
        vv_prep[h, :, :, :64] = v16[:, h * D_:(h + 1) * D_].reshape(
            c.KT, 128, D_).transpose(1, 0, 2)

    # Wfc_v[e, h*64+d] = sum_dd Wfc[e, h*64+dd] * Wv[dd, d]
    E, H, D = c.E, c.H, c.D
    wfcv = np.empty((E, E), np.float64)
    for h in range(H):
        wfcv[:, h * D:(h + 1) * D] = Wfc[:, h * D:(h + 1) * D] @ Wv
    # rhs tiles: wfc_prep[p, dt, e] = Wfc_v[e, dt*128+p]
    wfc_prep = np.ascontiguousarray(
        wfcv.T.reshape(c.ET, 128, E).transpose(1, 0, 2)).astype(np.float16)

    # w1_prep[zt, p, et*128 + z] = W1[zt*128+z, et*128+p]
    w1r = W1.reshape(c.ZT, 128, c.ET, 128)                 # [zt, z, et, p]
    w1_prep = np.ascontiguousarray(
        w1r.transpose(0, 3, 2, 1).reshape(c.ZT, 128, E)).astype(np.float16)

    # w2_prep[zt, p, e] = W2[e, zt*128+p]
    w2r = W2.T.reshape(c.ZT, 128, E)                       # [zt, p, e]
    w2_prep = np.ascontiguousarray(w2r).astype(np.float16)

    b1 = np.asarray(inputs["b1"], np.float32)
    b1_prep = np.ascontiguousarray(b1.reshape(c.ZT, 128).T)  # [128, ZT]

    return {
        "k1": k1_prep,
        "k2o": k2o_prep,
        "q1o": q1oT,
        "qq": qq_prep,
        # fc bias folded into the residual stream
        "qnat": (qs[perm] + np.asarray(inputs["bfc"], np.float32)[None, :]
                 ).astype(np.float16),
        "vv": vv_prep,
        "wfc": wfc_prep,
        "w1": w1_prep,
        "b1": b1_prep,
        "w2": w2_prep,
        "b2": np.asarray(inputs["b2"], np.float32)[None, :].astype(np.float16),
        "g1": np.asarray(inputs["ln1_g"], np.float16)[None, :],
        "be1": np.asarray(inputs["ln1_b"], np.float16)[None, :],
        "g2": np.asarray(inputs["ln2_g"], np.float16)[None, :],
        "be2": np.asarray(inputs["ln2_b"], np.float16)[None, :],
    }


_CACHE = {}


def kernel(**inputs):
    """Full-input entry point: shard across 8 cores, run, gather."""
    c = Cfg()
    B, S, E = inputs["queries"].shape
    assert (B, S, E) == (2, c.S, c.E), (B, S, E)

    if "nc" not in _CACHE:
        nc = build_nc(c)
        _split_waits(nc)   # walrus wait-slot workaround (compile path only)
        _CACHE["nc"] = nc
    nc = _CACHE["nc"]

    in_maps = [host_prep(c, inputs, core) for core in range(8)]

    from concourse.bass_utils import run_bass_kernel_spmd
    res = run_bass_kernel_spmd(nc, in_maps, core_ids=list(range(8)))

    perm = c.perm()
    out = np.empty((B, S, E), np.float32)
    cores_per_batch = 4
    for core in range(8):
        b = core // cores_per_batch
        slot = core % cores_per_batch
        block = np.empty((c.T, E), np.float32)
        block[perm] = res.results[core]["out"].astype(np.float32)
        out[b, slot * c.T:(slot + 1) * c.T] = block
    return out
